# revision 13
# baseline (speedup 1.0000x reference)
"""Triangular pairwise channel product on 8 Trainium2 NeuronCores.

out[b,h,w,k] = x[b,h,w,i_k] * x[b,h,w,j_k]  for the C*(C-1)/2 pairs
(i<j) in row-major (np.triu_indices) order.

Sharding: pure data parallel over batch - core c takes x[2c:2c+2].
Per core the 2*64*64 = 8192 spatial positions map to 128 SBUF
partitions (b_loc*64+h) x 64 groups (w).

d-offset formulation: for d in 1..63, prod_d[p,g,c] = x[p,g,c] *
x[p,g,c+d], c in [0, 64-d).  All operands are step-1 packed bf16 so
DVE's 2x_1p perf mode applies (2 elem/cycle).  Odd d reads its second
operand from x_odd (a one-channel-shifted DMA copy of x) to keep the
4B alignment 2x_1p requires.

v2 (stair-step): several consecutive same-parity d's share ONE
tensor_mul via a 4-level access pattern (partition, g, d, c): operand
a has d-stride 0 (broadcast), operand b d-stride 2 (overlapping
windows), out d-stride w_pad.  n=3 cuts DVE per-op overhead ~3x for
+6% rectangle-padding waste in SBUF/DMA bytes.  Measured baseline
facts: DVE tensor_tensor ~77ns/op marginal overhead, stores stream at
~433 GB/s/core on one HWDGE ring, ~7us fixed engine-barrier preamble,
~2.7us postamble.
"""

import numpy as np

import concourse.bacc as bacc
import concourse.bass as bass
import concourse.mybir as mybir
import concourse.tile as tile
from concourse.bass import AP
from concourse.bass_utils import run_bass_kernel_spmd

B, H, W, C = 16, 64, 64, 64
K = C * (C - 1) // 2  # 2016
N_CORES = 8
BP = B // N_CORES  # batch rows per core
P = BP * H         # 128 SBUF partitions
G = W              # position groups per partition
XLEN = G * C       # 4096 elements per partition
XPAD = 8           # tail pad: widest op row reads up to 4099
FP = mybir.dt.float32
BF = mybir.dt.bfloat16

# ---------------------------------------------------------------------------
# Stair-step plan: groups of n consecutive same-parity d's per DVE op.
# Group = (src_is_odd, d0, n, w_pad, qoff); row r handles d = d0 + 2r,
# covering c in [0, w_pad) (true width 64-d, rest is pad/waste).
# ---------------------------------------------------------------------------

def _make_plan(n: int = 3, extra_pad: int = 6):
    """extra_pad widens the last even group (single row d=62, w_pad 2)
    so K2 lands on a rounder number (2170 -> 2176): keeps the
    per-partition DRAM stride HBM-interleave friendly.  Its spill reads
    stay inside XPAD (62 + w_pad - 1 = 69 < 64 + XPAD)."""
    plan = []
    qoff = 0
    for parity in (0, 1):  # even d's first (they don't need x_odd)
        if parity == 0:
            ds = [d for d in range(2, C, 2)]
        else:
            ds = [d for d in range(1, C, 2)]
        i = 0
        while i < len(ds):
            grp = ds[i : i + n]
            w = C - grp[0]
            w_pad = w + (w % 2)
            if parity == 0 and i + n >= len(ds) and len(grp) == 1:
                w_pad += extra_pad
            plan.append((parity == 1, grp[0], len(grp), w_pad, qoff))
            qoff += len(grp) * w_pad
            i += n
    return plan, qoff


PLAN, K2 = _make_plan(3)
assert K2 == 2176, K2

# Groups computed on the (otherwise idle) GpSimd engine instead of DVE.
# Offloading the widest even block [2,4,6] (186 ch) drops DVE below the
# DMA store rate, making the pipeline purely store-bound.  GpSimd cost
# ~2.5 ns/elem => ~30 us busy, well within its idle window.
GP_GROUPS = {0}

# Ramped: small first tiles prime the store pipeline early (kills the
# DMA bubble waiting on a big tile-1 compute); small last tile cuts the
# DMA tail after the final multiply.
G_ITERS = [2, 4, 8, 10, 12, 12, 12, 4]
assert sum(G_ITERS) == G

# Host-side permutation: out[g, k] (triu pair k) -> flat device position.
# Device layout per g-iteration chunk (base g_off*K2): ops write contiguous
# blocks [qoff*Gi, (qoff + n*w_pad)*Gi), each block laid out (g, r, c).
_II, _JJ = np.triu_indices(C, k=1)
_qoff_d = np.zeros(C, dtype=np.int64)
_r_d = np.zeros(C, dtype=np.int64)
_wpad_d = np.zeros(C, dtype=np.int64)
_blk_d = np.zeros(C, dtype=np.int64)  # n*w_pad of d's group
for _odd, _d0, _n, _wp, _qo in PLAN:
    for _r in range(_n):
        _d = _d0 + 2 * _r
        _qoff_d[_d] = _qo
        _r_d[_d] = _r
        _wpad_d[_d] = _wp
        _blk_d[_d] = _n * _wp
_D = _JJ - _II  # [K]
_g_off_g = np.zeros(G, dtype=np.int64)
_Gi_g = np.zeros(G, dtype=np.int64)
_go = 0
for _Gi in G_ITERS:
    _g_off_g[_go : _go + _Gi] = _go
    _Gi_g[_go : _go + _Gi] = _Gi
    _go += _Gi
# IDX[g, k]: position in the [G*K2] flat per-partition output vector.
_gg = np.arange(G, dtype=np.int64)[:, None]
_IDX = (
    _g_off_g[:, None] * K2
    + _qoff_d[_D][None, :] * _Gi_g[:, None]
    + (_gg - _g_off_g[:, None]) * _blk_d[_D][None, :]
    + _r_d[_D][None, :] * _wpad_d[_D][None, :]
    + _II[None, :]
)

_nc_cache = None


def _op_aps(xt, xo, ot, g_off, Gi, grp):
    """Build (out_ap, a_ap, b_ap) for one stair-step group."""
    src_odd, d0, n, w_pad, qoff = grp
    xt_full = xt[:]
    xo_full = xo[:]
    ot_full = ot[:]
    xlen = xt_full.ap[0][0]
    olen = ot_full.ap[0][0]
    a = AP(xt_full.tensor, g_off * C,
           [[xlen, P], [C, Gi], [0, n], [1, w_pad]])
    if src_odd:
        b = AP(xo_full.tensor, g_off * C + d0 - 1,
               [[xlen, P], [C, Gi], [2, n], [1, w_pad]])
    else:
        b = AP(xt_full.tensor, g_off * C + d0,
               [[xlen, P], [C, Gi], [2, n], [1, w_pad]])
    out = AP(ot_full.tensor, qoff * Gi,
             [[olen, P], [n * w_pad, Gi], [w_pad, n], [1, w_pad]])
    return out, a, b


def build_stair(g_iters=None, bufs: int = 3) -> bass.Bass:
    nc = bacc.Bacc(
        "TRN2",
        target_bir_lowering=False,
        debug=False,
        num_devices=N_CORES,
    )
    if g_iters is None:
        g_iters = G_ITERS
    assert sum(g_iters) == G
    g0 = g_iters[0]

    x = nc.dram_tensor("x", [P, XLEN], BF, kind="ExternalInput")
    y = nc.dram_tensor("y", [P, G * K2], BF, kind="ExternalOutput")

    with tile.TileContext(nc) as tc:
        with (
            tc.tile_pool(name="xin", bufs=1) as xpool,
            tc.tile_pool(name="out", bufs=bufs) as opool,
        ):
            xt = xpool.tile([P, XLEN + XPAD], BF, tag="xt")
            xo = xpool.tile([P, XLEN + XPAD], BF, tag="xo")
            # Tail pads (read by the widest ops' spill columns).
            nc.gpsimd.memset(xt[:, XLEN : XLEN + XPAD], 0.0)
            nc.gpsimd.memset(xo[:, XLEN - 1 : XLEN + XPAD], 0.0)

            # Loads: chunk0 covers iter0's reads (incl. 4-elem spill into
            # group g0+1); bulk on the scalar ring in two chunks so iter1
            # isn't gated on the full input.
            c0 = (g0 + 2) * C  # iter0 reads < (g0+1)*64 + 4
            gm = 16 * C
            nc.sync.dma_start(out=xt[:, 0:c0], in_=x[:, 0:c0])
            nc.sync.dma_start(out=xo[:, 0 : c0 - 1], in_=x[:, 1:c0])
            nc.scalar.dma_start(out=xt[:, c0:gm], in_=x[:, c0:gm])
            nc.scalar.dma_start(out=xt[:, gm:XLEN], in_=x[:, gm:XLEN])
            nc.scalar.dma_start(out=xo[:, c0 - 1 : gm - 1], in_=x[:, c0:gm])
            nc.scalar.dma_start(out=xo[:, gm - 1 : XLEN - 1], in_=x[:, gm:XLEN])

            g_off = 0
            for it, Gi in enumerate(g_iters):
                ot = opool.tile([P, Gi * K2], BF, tag="ot")
                for gi, grp in enumerate(PLAN):
                    out, a, b = _op_aps(xt, xo, ot, g_off, Gi, grp)
                    if gi in GP_GROUPS:
                        nc.gpsimd.tensor_mul(out, a, b)
                    else:
                        nc.vector.tensor_mul(out, a, b)
                nc.sync.dma_start(
                    out=y[:, g_off * K2 : (g_off + Gi) * K2],
                    in_=ot[:],
                )
                g_off += Gi

    nc.finalize()
    return nc


def make_in_maps(x: np.ndarray) -> list[dict[str, np.ndarray]]:
    import ml_dtypes

    x = np.ascontiguousarray(x, dtype=np.float32).astype(ml_dtypes.bfloat16)
    return [
        {"x": x[c * BP : (c + 1) * BP].reshape(P, XLEN)} for c in range(N_CORES)
    ]


def kernel(**inputs: np.ndarray) -> np.ndarray:
    global _nc_cache
    if _nc_cache is None:
        _nc_cache = build_stair()
    res = run_bass_kernel_spmd(
        _nc_cache, make_in_maps(inputs["inputs"]), list(range(N_CORES))
    ).results
    ypad = np.concatenate(
        [
            np.asarray(res[c]["y"]).reshape(BP, H, G * K2)
            for c in range(N_CORES)
        ],
        axis=0,
    )
    # Undo the stair-step block layout -> [W, triu (i,j)] + upcast.
    return np.take(ypad, _IDX, axis=-1).astype(np.float32)


# revision 15
# speedup vs baseline: 1.3873x; 1.3873x over previous
"""Triangular pairwise channel product on 8 Trainium2 NeuronCores.

out[b,h,w,k] = x[b,h,w,i_k] * x[b,h,w,j_k]  for the C*(C-1)/2 pairs
(i<j) in row-major (np.triu_indices) order.

Sharding: pure data parallel over batch - core c takes x[2c:2c+2].
Per core the 2*64*64 = 8192 spatial positions map to 128 SBUF
partitions (b_loc*64+h) x 64 groups (w).

d-offset formulation: for d in 1..63, prod_d[p,g,c] = x[p,g,c] *
x[p,g,c+d], c in [0, 64-d).  All operands are step-1 packed bf16 so
DVE's 2x_1p perf mode applies (2 elem/cycle).  Odd d reads its second
operand from x_odd (a one-channel-shifted DMA copy of x) to keep the
4B alignment 2x_1p requires.

v2 (stair-step): several consecutive same-parity d's share ONE
tensor_mul via a 4-level access pattern (partition, g, d, c): operand
a has d-stride 0 (broadcast), operand b d-stride 2 (overlapping
windows), out d-stride w_pad.  n=3 cuts DVE per-op overhead ~3x for
+6% rectangle-padding waste in SBUF/DMA bytes.  Measured baseline
facts: DVE tensor_tensor ~77ns/op marginal overhead, stores stream at
~433 GB/s/core on one HWDGE ring, ~7us fixed engine-barrier preamble,
~2.7us postamble.
"""

import numpy as np

import concourse.bacc as bacc
import concourse.bass as bass
import concourse.mybir as mybir
import concourse.tile as tile
from concourse.bass import AP
from concourse.bass_utils import run_bass_kernel_spmd

B, H, W, C = 16, 64, 64, 64
K = C * (C - 1) // 2  # 2016
N_CORES = 8
BP = B // N_CORES  # batch rows per core
P = BP * H         # 128 SBUF partitions
G = W              # position groups per partition
XLEN = G * C       # 4096 elements per partition
XPAD = 8           # tail pad: widest op row reads up to 4099
FP = mybir.dt.float32
BF = mybir.dt.bfloat16

# ---------------------------------------------------------------------------
# Stair-step plan: groups of n consecutive same-parity d's per DVE op.
# Group = (src_is_odd, d0, n, w_pad, qoff); row r handles d = d0 + 2r,
# covering c in [0, w_pad) (true width 64-d, rest is pad/waste).
# ---------------------------------------------------------------------------

def _make_plan(n: int = 3, extra_pad: int = 6):
    """extra_pad widens the last even group (single row d=62, w_pad 2)
    so K2 lands on a rounder number (2170 -> 2176): keeps the
    per-partition DRAM stride HBM-interleave friendly.  Its spill reads
    stay inside XPAD (62 + w_pad - 1 = 69 < 64 + XPAD)."""
    plan = []
    qoff = 0
    for parity in (0, 1):  # even d's first (they don't need x_odd)
        if parity == 0:
            ds = [d for d in range(2, C, 2)]
        else:
            ds = [d for d in range(1, C, 2)]
        i = 0
        while i < len(ds):
            grp = ds[i : i + n]
            w = C - grp[0]
            w_pad = w + (w % 2)
            if parity == 0 and i + n >= len(ds) and len(grp) == 1:
                w_pad += extra_pad
            plan.append((parity == 1, grp[0], len(grp), w_pad, qoff))
            qoff += len(grp) * w_pad
            i += n
    return plan, qoff


PLAN, K2 = _make_plan(3)
assert K2 == 2176, K2

# GpSimd offload was tried and regressed badly (one op took 45 us and
# concurrent SBUF traffic slowed DVE ~14%): engines fight over SBUF
# bandwidth.  Keep everything on DVE.
GP_GROUPS: set[int] = set()

# Ramped: small first tiles prime the store pipeline early (kills the
# DMA bubble waiting on a big tile-1 compute); small last tile cuts the
# DMA tail after the final multiply.
G_ITERS = [2, 4, 8, 10, 12, 12, 12, 4]
assert sum(G_ITERS) == G

# Host-side permutation: out[g, k] (triu pair k) -> flat device position.
# Device layout per g-iteration chunk (base g_off*K2): ops write contiguous
# blocks [qoff*Gi, (qoff + n*w_pad)*Gi), each block laid out (g, r, c).
_II, _JJ = np.triu_indices(C, k=1)
_qoff_d = np.zeros(C, dtype=np.int64)
_r_d = np.zeros(C, dtype=np.int64)
_wpad_d = np.zeros(C, dtype=np.int64)
_blk_d = np.zeros(C, dtype=np.int64)  # n*w_pad of d's group
for _odd, _d0, _n, _wp, _qo in PLAN:
    for _r in range(_n):
        _d = _d0 + 2 * _r
        _qoff_d[_d] = _qo
        _r_d[_d] = _r
        _wpad_d[_d] = _wp
        _blk_d[_d] = _n * _wp
_D = _JJ - _II  # [K]
_g_off_g = np.zeros(G, dtype=np.int64)
_Gi_g = np.zeros(G, dtype=np.int64)
_go = 0
for _Gi in G_ITERS:
    _g_off_g[_go : _go + _Gi] = _go
    _Gi_g[_go : _go + _Gi] = _Gi
    _go += _Gi
# IDX[g, k]: position in the [G*K2] flat per-partition output vector.
_gg = np.arange(G, dtype=np.int64)[:, None]
_IDX = (
    _g_off_g[:, None] * K2
    + _qoff_d[_D][None, :] * _Gi_g[:, None]
    + (_gg - _g_off_g[:, None]) * _blk_d[_D][None, :]
    + _r_d[_D][None, :] * _wpad_d[_D][None, :]
    + _II[None, :]
)

_nc_cache = None


def _op_aps(xt, xo, ot, g_off, Gi, grp):
    """Build (out_ap, a_ap, b_ap) for one stair-step group."""
    src_odd, d0, n, w_pad, qoff = grp
    xt_full = xt[:]
    xo_full = xo[:]
    ot_full = ot[:]
    xlen = xt_full.ap[0][0]
    olen = ot_full.ap[0][0]
    a = AP(xt_full.tensor, g_off * C,
           [[xlen, P], [C, Gi], [0, n], [1, w_pad]])
    if src_odd:
        b = AP(xo_full.tensor, g_off * C + d0 - 1,
               [[xlen, P], [C, Gi], [2, n], [1, w_pad]])
    else:
        b = AP(xt_full.tensor, g_off * C + d0,
               [[xlen, P], [C, Gi], [2, n], [1, w_pad]])
    out = AP(ot_full.tensor, qoff * Gi,
             [[olen, P], [n * w_pad, Gi], [w_pad, n], [1, w_pad]])
    return out, a, b


def build_stair(g_iters=None, bufs: int = 3) -> bass.Bass:
    nc = bacc.Bacc(
        "TRN2",
        target_bir_lowering=False,
        debug=False,
        num_devices=N_CORES,
    )
    if g_iters is None:
        g_iters = G_ITERS
    assert sum(g_iters) == G
    g0 = g_iters[0]

    x = nc.dram_tensor("x", [P, XLEN], BF, kind="ExternalInput")
    y = nc.dram_tensor("y", [P, G * K2], BF, kind="ExternalOutput")

    with tile.TileContext(nc) as tc:
        with (
            tc.tile_pool(name="xin", bufs=1) as xpool,
            tc.tile_pool(name="out", bufs=bufs) as opool,
        ):
            xt = xpool.tile([P, XLEN + XPAD], BF, tag="xt")
            xo = xpool.tile([P, XLEN + XPAD], BF, tag="xo")
            # Tail pads (read by the widest ops' spill columns).
            nc.gpsimd.memset(xt[:, XLEN : XLEN + XPAD], 0.0)
            nc.gpsimd.memset(xo[:, XLEN - 1 : XLEN + XPAD], 0.0)

            # Loads: chunk0 covers iter0's reads (incl. 4-elem spill into
            # group g0+1); bulk on the scalar ring in two chunks so iter1
            # isn't gated on the full input.
            c0 = (g0 + 2) * C  # iter0 reads < (g0+1)*64 + 4
            gm = 16 * C
            nc.sync.dma_start(out=xt[:, 0:c0], in_=x[:, 0:c0])
            nc.sync.dma_start(out=xo[:, 0 : c0 - 1], in_=x[:, 1:c0])
            nc.scalar.dma_start(out=xt[:, c0:gm], in_=x[:, c0:gm])
            nc.scalar.dma_start(out=xt[:, gm:XLEN], in_=x[:, gm:XLEN])
            nc.scalar.dma_start(out=xo[:, c0 - 1 : gm - 1], in_=x[:, c0:gm])
            nc.scalar.dma_start(out=xo[:, gm - 1 : XLEN - 1], in_=x[:, gm:XLEN])

            g_off = 0
            for it, Gi in enumerate(g_iters):
                ot = opool.tile([P, Gi * K2], BF, tag="ot")
                for gi, grp in enumerate(PLAN):
                    out, a, b = _op_aps(xt, xo, ot, g_off, Gi, grp)
                    if gi in GP_GROUPS:
                        nc.gpsimd.tensor_mul(out, a, b)
                    else:
                        nc.vector.tensor_mul(out, a, b)
                # Split each store across both HWDGE rings: tests whether
                # the ~27 GB/s/engine store rate is queue-limited.
                mid = (Gi * K2) // 2
                nc.sync.dma_start(
                    out=y[:, g_off * K2 : g_off * K2 + mid],
                    in_=ot[:, 0:mid],
                )
                nc.scalar.dma_start(
                    out=y[:, g_off * K2 + mid : (g_off + Gi) * K2],
                    in_=ot[:, mid:],
                )
                g_off += Gi

    nc.finalize()
    return nc


def make_in_maps(x: np.ndarray) -> list[dict[str, np.ndarray]]:
    import ml_dtypes

    x = np.ascontiguousarray(x, dtype=np.float32).astype(ml_dtypes.bfloat16)
    return [
        {"x": x[c * BP : (c + 1) * BP].reshape(P, XLEN)} for c in range(N_CORES)
    ]


def kernel(**inputs: np.ndarray) -> np.ndarray:
    global _nc_cache
    if _nc_cache is None:
        _nc_cache = build_stair()
    res = run_bass_kernel_spmd(
        _nc_cache, make_in_maps(inputs["inputs"]), list(range(N_CORES))
    ).results
    ypad = np.concatenate(
        [
            np.asarray(res[c]["y"]).reshape(BP, H, G * K2)
            for c in range(N_CORES)
        ],
        axis=0,
    )
    # Undo the stair-step block layout -> [W, triu (i,j)] + upcast.
    return np.take(ypad, _IDX, axis=-1).astype(np.float32)
